# revision 1
# baseline (speedup 1.0000x reference)
"""Trainium2 Bass kernel: per-batch grouped Conv2d (16 batches, 1->32 ch, 9x9, pad=3).

Pure data parallel: 2 batches per core on 8 NeuronCores.  Per batch:
  out[j, y, x] = sum_{ky,kx} W[j,ky,kx] * xpad[y+ky, x+kx]
computed as 9 PSUM-accumulated matmuls (one per kernel column dx) with
contraction over 12 padded input rows (K=12).  One output block = 4 output
rows x 32 channels = 128 PSUM partitions x 510 columns.  Four PE row-strips
(tile_position (0|32|64|96, 0)) run 4 blocks concurrently; fp32r matmuls
stream at ~1 col/cycle.  DVE/ACT evacuate PSUM -> SBUF, HWDGE DMA stores.
"""

import numpy as np

import concourse.bacc as bacc
import concourse.mybir as mybir
from concourse.bass_utils import run_bass_kernel_spmd
from concourse.tile import TileContext

B, J, KH, KW = 16, 32, 9, 9
H = W_IN = 512
PAD = 3          # int(9/2) - 1
HO = WO = 510    # 512 + 2*3 - 9 + 1
NCORES = 8
BPC = B // NCORES          # batches per core = 2
XP = 520                   # padded row length: 3 + 512 + 5
XR = 524                   # padded rows: 3 + 512 + 9 (strip slicing headroom)
NROUND = 32                # 32 rounds x 4 strips x 4 rows = 512 out rows (last 2 dropped)

DT = mybir.dt.float32
DTR = mybir.dt.float32r

_PROG_CACHE = {}


def _build_program(repeat=1, timing=False):
    nc = bacc.Bacc("TRN2", target_bir_lowering=False, debug=False,
                   num_devices=NCORES)
    xpad = nc.dram_tensor("xpad", [BPC, XR, XP], DTR, kind="ExternalInput")
    wprep = nc.dram_tensor("wprep", [BPC, 12, KW, 128], DTR, kind="ExternalInput")
    if timing:
        # timing build: full-size result stays in device DRAM; only a tiny
        # tensor is transferred back, so wall-clock deltas isolate HW time.
        out = nc.dram_tensor("out_scratch", [BPC, J, HO, WO], DT)
        dummy = nc.dram_tensor("tdummy", [1, 128], DT, kind="ExternalOutput")
    else:
        out = nc.dram_tensor("out", [BPC, J, HO, WO], DT, kind="ExternalOutput")

    with TileContext(nc) as tc:
        with (
            tc.tile_pool(name="wpool", bufs=1) as wpool,
            tc.tile_pool(name="imgpool", bufs=2) as imgpool,
            tc.tile_pool(name="pspool", bufs=2, space="PSUM") as pspool,
            tc.tile_pool(name="evpool", bufs=3) as evpool,
            tc.tile_pool(name="scrpool", bufs=2, space="DRAM") as scrpool,
        ):
            # Stationary weight tiles, replicated on all 4 strips.
            # Per strip s (partitions 32s..32s+11):
            #   free [b*1152 + dx*128 + m] = wprep[b, dy', dx, m]
            wt = wpool.tile([128, BPC * KW * 128], DTR)
            for s in range(4):
                for b in range(BPC):
                    nc.sync.dma_start(
                        out=wt[32 * s:32 * s + 12,
                               b * KW * 128:(b + 1) * KW * 128],
                        in_=wprep[b].rearrange("p a m -> p (a m)"),
                    )

            for _ in range(repeat):
                for b in range(BPC):
                    # Image row panels: strip s, slot j holds padded rows
                    # 16j+4s+dy' (dy'=0..11) at free offset j*520.
                    img = imgpool.tile([128, NROUND * XP], DTR)
                    for s in range(4):
                        src = xpad[b, 4 * s:4 * s + 512, :] \
                            .rearrange("(j p) x -> p j x", p=16)[0:12]
                        nc.sync.dma_start(
                            out=img[32 * s:32 * s + 12, :]
                                .rearrange("p (j x) -> p j x", x=XP),
                            in_=src,
                        )

                    for j in range(NROUND):
                        pss = [pspool.tile([128, WO], DT, tag=f"ps{s}",
                                           name=f"ps{s}_{b}_{j}")
                               for s in range(4)]
                        for dx in range(KW):
                            for s in range(4):
                                lhsT = wt[32 * s:32 * s + 12,
                                          b * KW * 128 + dx * 128:
                                          b * KW * 128 + (dx + 1) * 128]
                                rhs = img[32 * s:32 * s + 12,
                                          j * XP + dx:j * XP + dx + WO]
                                nc.tensor.matmul(
                                    pss[s][:], lhsT, rhs,
                                    start=(dx == 0), stop=(dx == KW - 1),
                                    tile_position=(32 * s, 0),
                                )
                        ev = evpool.tile([128, 4 * WO], DT)
                        for s in range(4):
                            if s == 3:
                                nc.scalar.copy(ev[:, s * WO:(s + 1) * WO],
                                               pss[s][:])
                            else:
                                nc.vector.tensor_copy(ev[:, s * WO:(s + 1) * WO],
                                                      pss[s][:])
                        # store: rows 16j+4s .. +3; partition p = ch*4 + sy.
                        # src stays [128, 510]; the DMA balancer splits the
                        # partition dim against the [32, 4, 510] DRAM dest.
                        for s in range(4):
                            src2 = ev[:, s * WO:(s + 1) * WO]
                            if j < NROUND - 1 or s < 3:
                                nc.sync.dma_start(
                                    out=out[b, :, 16 * j + 4 * s:
                                            16 * j + 4 * s + 4, :],
                                    in_=src2,
                                )
                            else:
                                # block (31,3) covers rows 508..511; keep
                                # 508/509 via DRAM bounce (sy-subset of the
                                # partition dim is not a rectangular AP).
                                scr = scrpool.tile([J, 4, WO], DT)
                                nc.sync.dma_start(out=scr[:], in_=src2)
                                nc.gpsimd.dma_start(
                                    out=out[b, :, 508:510, :],
                                    in_=scr[:, 0:2, :],
                                )
            if timing:
                nc.sync.dma_start(out=dummy[:], in_=wt[0:1, 0:128].bitcast(DT))
    nc.compile()
    return nc


def _get_program(repeat=1, timing=False):
    key = (repeat, timing)
    if key not in _PROG_CACHE:
        _PROG_CACHE[key] = _build_program(repeat, timing)
    return _PROG_CACHE[key]


def _prep_core_inputs(input, weight, c):
    xp = np.zeros((BPC, XR, XP), np.float32)
    xp[:, PAD:PAD + H, PAD:PAD + W_IN] = input[BPC * c:BPC * (c + 1), 0]
    wp = np.zeros((BPC, 12, KW, 128), np.float32)
    wsl = weight[BPC * c:BPC * (c + 1)]            # [2, 32, 9, 9]
    wq = wsl.transpose(0, 2, 3, 1)                 # [2, ky, kx, j]
    for sy in range(4):
        # wp[b, sy+ky, dx, j*4+sy] = W[b, j, ky, dx]
        wp[:, sy:sy + 9, :, sy::4] = wq
    return {"xpad": xp, "wprep": wp}


def kernel(input, weight, _repeat=1, _timing=False):
    input = np.ascontiguousarray(np.asarray(input, np.float32))
    weight = np.ascontiguousarray(np.asarray(weight, np.float32))
    nc = _get_program(_repeat, _timing)
    in_maps = [_prep_core_inputs(input, weight, c) for c in range(NCORES)]
    res = run_bass_kernel_spmd(nc, in_maps, list(range(NCORES)))
    if _timing:
        return None
    outs = np.stack([res.results[c]["out"] for c in range(NCORES)])
    return outs.reshape(B, J, HO, WO).astype(np.float32, copy=False)



# revision 2
# speedup vs baseline: 1.4274x; 1.4274x over previous
"""Trainium2 Bass kernel v2: per-batch grouped Conv2d (16 batches, 1->32ch, 9x9, pad=3).

Data parallel: 2 batches/core on 8 cores.  Per batch, one matmul per output
block of 4 rows x 32 ch (= 128 PSUM partitions x 510 cols), contraction
K=108 = 12 row-offsets x 9 kernel-cols over a 9x column-shift-replicated
bf16 image in SBUF:
  X9[12g+r, t*510+x] = xpad[y0(t)+r, x+g]          (bf16)
  lhsT[12g+r, 4ch+sy] = W[ch, r-sy, g]             (bf16, 0 outside 0..8)
  psum[(ch,sy), x] = sum_{g,r} lhsT * X9 = conv_out[ch, y0+sy, x]
256 matmuls/core (vs 2304 in v1); ~620 static instructions.  Output is
stored bf16 in a device-native [b, half, grp, 128, 2040] layout (fully
contiguous 1MB-class stores); the host transposes back to NCHW fp32.
Timing builds wrap the body in a For_i hardware loop so the repeat-delta
measures steady-state device time, not NEFF reload overhead.
"""

import numpy as np
import ml_dtypes

import concourse.bacc as bacc
import concourse.mybir as mybir
from concourse.bass_utils import run_bass_kernel_spmd
from concourse.tile import TileContext

B, J, KH, KW = 16, 32, 9, 9
H = W_IN = 512
PAD = 3
HO = WO = 510
NCORES = 8
BPC = 2
XPR, XPC = 520, 520           # padded image rows/cols in DRAM
NB = 64                       # blocks per half-batch
NG = NB // 4                  # store groups per half-batch
KC = 108                      # contraction: 12 row-offsets x 9 kx

DT = mybir.dt.float32
BF = mybir.dt.bfloat16
BF_NP = ml_dtypes.bfloat16

_PROG_CACHE = {}


def _win(anchor, dims):
    """Arbitrary strided view: override the AP dims of an anchor slice
    (keeps its element offset).  dims = [(step_elems, nelems), ...],
    outermost first; dim 0 is the partition dim for SBUF APs."""
    v = anchor.copy()
    v.ap = mybir.VecI64Pair(dims)
    return v


def _emit_body(nc, tc, pools, xpad, wt, out):
    x9pool, pspool, evpool = pools
    for b in range(BPC):
        for half in range(2):
            yoff = 254 * half
            x9 = x9pool.tile([KC, NB * WO], BF)
            for g in range(KW):
                # X9[12g+r, t*510+x] = xpad[b, yoff+4t+r, x+g]
                src = _win(xpad[b, yoff:yoff + 1, g:g + 1],
                           [(XPC, 12), (4 * XPC, NB), (1, WO)])
                nc.sync.dma_start(
                    out=x9[12 * g:12 * g + 12, :]
                        .rearrange("r (t x) -> r t x", x=WO),
                    in_=src,
                )
            for t in range(NB):
                ps = pspool.tile([128, WO], DT, tag="ps")
                nc.tensor.matmul(ps[:], wt[:, 128 * b:128 * (b + 1)],
                                 x9[:, t * WO:(t + 1) * WO],
                                 start=True, stop=True)
                u = t % 4
                if u == 0:
                    ev = evpool.tile([128, 4 * WO], BF)
                nc.vector.tensor_copy(ev[:, u * WO:(u + 1) * WO], ps[:])
                if u == 3:
                    nc.sync.dma_start(out=out[b, half, t // 4], in_=ev[:])


def _build_program(repeat=1, timing=False):
    nc = bacc.Bacc("TRN2", target_bir_lowering=False, debug=False,
                   num_devices=NCORES)
    xpad = nc.dram_tensor("xpad", [BPC, XPR, XPC], BF, kind="ExternalInput")
    wprep = nc.dram_tensor("wprep", [BPC, KC, 128], BF, kind="ExternalInput")
    if timing:
        out = nc.dram_tensor("out_scratch", [BPC, 2, NG, 128, 4 * WO], BF)
        dummy = nc.dram_tensor("tdummy", [1, 64], DT, kind="ExternalOutput")
    else:
        out = nc.dram_tensor("out", [BPC, 2, NG, 128, 4 * WO], BF,
                             kind="ExternalOutput")

    with TileContext(nc) as tc:
        with (
            tc.tile_pool(name="wpool", bufs=1) as wpool,
            tc.tile_pool(name="x9pool", bufs=2) as x9pool,
            tc.tile_pool(name="pspool", bufs=8, space="PSUM") as pspool,
            tc.tile_pool(name="evpool", bufs=3) as evpool,
        ):
            wt = wpool.tile([KC, BPC * 128], BF)
            for b in range(BPC):
                nc.sync.dma_start(out=wt[:, 128 * b:128 * (b + 1)],
                                  in_=wprep[b])
            pools = (x9pool, pspool, evpool)
            if timing and repeat > 1:
                with tc.For_i(0, repeat, 1):
                    _emit_body(nc, tc, pools, xpad, wt, out)
            else:
                _emit_body(nc, tc, pools, xpad, wt, out)
            if timing:
                nc.sync.dma_start(out=dummy[:], in_=wt[0:1, 0:128].bitcast(DT))
    nc.compile()
    return nc


def _get_program(repeat=1, timing=False):
    key = (repeat, timing)
    if key not in _PROG_CACHE:
        _PROG_CACHE[key] = _build_program(repeat, timing)
    return _PROG_CACHE[key]


def _prep_core_inputs(input, weight, c):
    xp = np.zeros((BPC, XPR, XPC), BF_NP)
    xp[:, PAD:PAD + H, PAD:PAD + W_IN] = input[BPC * c:BPC * (c + 1), 0] \
        .astype(BF_NP)
    wp = np.zeros((BPC, KC, 128), BF_NP)
    wsl = weight[BPC * c:BPC * (c + 1)].astype(BF_NP)   # [2, 32, 9, 9]
    for g in range(KW):
        for sy in range(4):
            # wp[b, 12g + sy + ky, 4ch + sy] = W[b, ch, ky, g]
            wp[:, 12 * g + sy:12 * g + sy + KH, sy::4] = \
                wsl[:, :, :, g].transpose(0, 2, 1)
    return {"xpad": xp, "wprep": wp}


def _assemble(dev_out):
    """dev_out: [BPC, 2, NG, 128, 2040] bf16 -> [BPC, J, HO, WO] f32.
    dev[b, h, gl, 4ch+sy, 510u+x] = out[b, ch, 254h + 16gl + 4u + sy, x]."""
    full = np.empty((BPC, J, HO, WO), np.float32)
    d = np.asarray(dev_out).reshape(BPC, 2, NG, J, 4, 4, WO)
    for h in range(2):
        # d[:, h] axes (b, gl, ch, sy, u, x); row-within-half = 16gl+4u+sy
        blk = d[:, h].transpose(0, 2, 1, 4, 3, 5).reshape(BPC, J, 256, WO) \
            .astype(np.float32)
        full[:, :, 254 * h:254 * h + 256, :] = blk
    return full


def kernel(input, weight, _repeat=1, _timing=False):
    input = np.ascontiguousarray(np.asarray(input, np.float32))
    weight = np.ascontiguousarray(np.asarray(weight, np.float32))
    nc = _get_program(_repeat, _timing)
    in_maps = [_prep_core_inputs(input, weight, c) for c in range(NCORES)]
    res = run_bass_kernel_spmd(nc, in_maps, list(range(NCORES)))
    if _timing:
        return None
    outs = np.stack([_assemble(res.results[c]["out"]) for c in range(NCORES)])
    return outs.reshape(B, J, HO, WO)


# revision 3
# speedup vs baseline: 1.4343x; 1.0048x over previous
"""Trainium2 Bass kernel v2: per-batch grouped Conv2d (16 batches, 1->32ch, 9x9, pad=3).

Data parallel: 2 batches/core on 8 cores.  Per batch, one matmul per output
block of 4 rows x 32 ch (= 128 PSUM partitions x 510 cols), contraction
K=108 = 12 row-offsets x 9 kernel-cols over a 9x column-shift-replicated
bf16 image in SBUF:
  X9[12g+r, t*510+x] = xpad[y0(t)+r, x+g]          (bf16)
  lhsT[12g+r, 4ch+sy] = W[ch, r-sy, g]             (bf16, 0 outside 0..8)
  psum[(ch,sy), x] = sum_{g,r} lhsT * X9 = conv_out[ch, y0+sy, x]
256 matmuls/core (vs 2304 in v1); ~620 static instructions.  Output is
stored bf16 in a device-native [b, half, grp, 128, 2040] layout (fully
contiguous 1MB-class stores); the host transposes back to NCHW fp32.
Timing builds wrap the body in a For_i hardware loop so the repeat-delta
measures steady-state device time, not NEFF reload overhead.
"""

import numpy as np
import ml_dtypes

import concourse.bacc as bacc
import concourse.mybir as mybir
from concourse.bass_utils import run_bass_kernel_spmd
from concourse.tile import TileContext

B, J, KH, KW = 16, 32, 9, 9
H = W_IN = 512
PAD = 3
HO = WO = 510
NCORES = 8
BPC = 2
XPR, XPC = 520, 520           # padded image rows/cols in DRAM
NB = 64                       # blocks per half-batch
NG = NB // 4                  # store groups per half-batch
KC = 108                      # contraction: 12 row-offsets x 9 kx

DT = mybir.dt.float32
BF = mybir.dt.bfloat16
BF_NP = ml_dtypes.bfloat16

_PROG_CACHE = {}


def _win(anchor, dims):
    """Arbitrary strided view: override the AP dims of an anchor slice
    (keeps its element offset).  dims = [(step_elems, nelems), ...],
    outermost first; dim 0 is the partition dim for SBUF APs."""
    v = anchor.copy()
    v.ap = mybir.VecI64Pair(dims)
    return v


def _emit_body(nc, tc, pools, xpad, wt, out):
    x9pool, pspool, evpool = pools
    for b in range(BPC):
        for half in range(2):
            yoff = 254 * half
            x9 = x9pool.tile([KC, NB * WO], BF)
            x9g = x9.rearrange("(r g) f -> g r f", g=KW)
            for g in range(KW):
                # X9[9r+g, t*510+x] = xpad[b, yoff+4t+r, x+g].
                # Partition stride 9 spreads each load across all SBUF
                # ports (contiguous 12-partition tiles only engage ~3 of
                # 16 SDMA engines).
                src = _win(xpad[b, yoff:yoff + 1, g:g + 1],
                           [(XPC, 12), (4 * XPC, NB), (1, WO)])
                nc.sync.dma_start(
                    out=x9g[g].rearrange("r (t x) -> r t x", x=WO),
                    in_=src,
                )
            for t in range(NB):
                ps = pspool.tile([128, WO], DT, tag="ps")
                nc.tensor.matmul(ps[:], wt[:, 128 * b:128 * (b + 1)],
                                 x9[:, t * WO:(t + 1) * WO],
                                 start=True, stop=True)
                u = t % 4
                if u == 0:
                    ev = evpool.tile([128, 4 * WO], BF)
                nc.vector.tensor_copy(ev[:, u * WO:(u + 1) * WO], ps[:])
                if u == 3:
                    nc.sync.dma_start(out=out[b, half, t // 4], in_=ev[:])


def _build_program(repeat=1, timing=False):
    nc = bacc.Bacc("TRN2", target_bir_lowering=False, debug=False,
                   num_devices=NCORES)
    xpad = nc.dram_tensor("xpad", [BPC, XPR, XPC], BF, kind="ExternalInput")
    wprep = nc.dram_tensor("wprep", [BPC, KC, 128], BF, kind="ExternalInput")
    if timing:
        out = nc.dram_tensor("out_scratch", [BPC, 2, NG, 128, 4 * WO], BF)
        dummy = nc.dram_tensor("tdummy", [1, 64], DT, kind="ExternalOutput")
    else:
        out = nc.dram_tensor("out", [BPC, 2, NG, 128, 4 * WO], BF,
                             kind="ExternalOutput")

    with TileContext(nc) as tc:
        with (
            tc.tile_pool(name="wpool", bufs=1) as wpool,
            tc.tile_pool(name="x9pool", bufs=2) as x9pool,
            tc.tile_pool(name="pspool", bufs=8, space="PSUM") as pspool,
            tc.tile_pool(name="evpool", bufs=3) as evpool,
        ):
            wt = wpool.tile([KC, BPC * 128], BF)
            for b in range(BPC):
                nc.sync.dma_start(out=wt[:, 128 * b:128 * (b + 1)],
                                  in_=wprep[b])
            pools = (x9pool, pspool, evpool)
            if timing and repeat > 1:
                with tc.For_i(0, repeat, 1):
                    _emit_body(nc, tc, pools, xpad, wt, out)
            else:
                _emit_body(nc, tc, pools, xpad, wt, out)
            if timing:
                nc.sync.dma_start(out=dummy[:], in_=wt[0:1, 0:128].bitcast(DT))
    nc.compile()
    return nc


def _get_program(repeat=1, timing=False):
    key = (repeat, timing)
    if key not in _PROG_CACHE:
        _PROG_CACHE[key] = _build_program(repeat, timing)
    return _PROG_CACHE[key]


def _prep_core_inputs(input, weight, c):
    xp = np.zeros((BPC, XPR, XPC), BF_NP)
    xp[:, PAD:PAD + H, PAD:PAD + W_IN] = input[BPC * c:BPC * (c + 1), 0] \
        .astype(BF_NP)
    wp = np.zeros((BPC, KC, 128), BF_NP)
    wsl = weight[BPC * c:BPC * (c + 1)].astype(BF_NP)   # [2, 32, 9, 9]
    for g in range(KW):
        for sy in range(4):
            # wp[b, 9*(sy+ky) + g, 4ch + sy] = W[b, ch, ky, g]
            wp[:, 9 * sy + g:9 * sy + g + 9 * KH:KW, sy::4] = \
                wsl[:, :, :, g].transpose(0, 2, 1)
    return {"xpad": xp, "wprep": wp}


def _assemble(dev_out):
    """dev_out: [BPC, 2, NG, 128, 2040] bf16 -> [BPC, J, HO, WO] f32.
    dev[b, h, gl, 4ch+sy, 510u+x] = out[b, ch, 254h + 16gl + 4u + sy, x]."""
    full = np.empty((BPC, J, HO, WO), np.float32)
    d = np.asarray(dev_out).reshape(BPC, 2, NG, J, 4, 4, WO)
    for h in range(2):
        # d[:, h] axes (b, gl, ch, sy, u, x); row-within-half = 16gl+4u+sy
        blk = d[:, h].transpose(0, 2, 1, 4, 3, 5).reshape(BPC, J, 256, WO) \
            .astype(np.float32)
        full[:, :, 254 * h:254 * h + 256, :] = blk
    return full


def kernel(input, weight, _repeat=1, _timing=False):
    input = np.ascontiguousarray(np.asarray(input, np.float32))
    weight = np.ascontiguousarray(np.asarray(weight, np.float32))
    nc = _get_program(_repeat, _timing)
    in_maps = [_prep_core_inputs(input, weight, c) for c in range(NCORES)]
    res = run_bass_kernel_spmd(nc, in_maps, list(range(NCORES)))
    if _timing:
        return None
    outs = np.stack([_assemble(res.results[c]["out"]) for c in range(NCORES)])
    return outs.reshape(B, J, HO, WO)
